# revision 10
# baseline (speedup 1.0000x reference)
"""MultiHeadRoPE kernel for Trainium2 (8 NeuronCores, batch-sharded).

Computes, per batch b:
    q_r = rope(q[b]); k_r = rope(k[b]); v_r = rope(v[b])
    out[b] = (q_r @ k_r.T) * (1/sqrt(D)) @ v_r
reassociated (no softmax present) as
    out[b] = q_r @ (scale * (k_r.T @ v_r))
turning an O(S^2 D) problem with a 64MB intermediate into an O(S D^2)
memory-bound one.

Device layout ("div-layout"): a [S, D] = [4096, 64] tensor is stored in
SBUF/DRAM as [128, 2048] where partition p holds tokens 32p..32p+31
contiguously -> every DMA moves 8KB contiguous per partition.

RoPE: out = x*C + sw(x)*S' (sw = pair swap; S' sign-folded). All rope on
DVE only: GPSIMD shares DVE's second SBUF port, so concurrent gpsimd
elementwise halves both engines. Work is split into 2 groups per tensor
and DMAs are split to match, so PE matmuls pipeline behind rope groups.

Matmul A (W = k_r.T @ v_r) runs as 16 pair-packed [M=128,K=128,N=128]
fp32 matmuls: lhsT = [k_r(2p) | k_r(2p+1)], rhs = [v_r(2p) | v_r(2p+1)]
accumulating into one [128,128] PSUM tile whose diagonal 64x64 quadrants
hold the two true partial products (off-diagonal quadrants are unused
cross terms). A DVE add folds the quadrants; ACT builds a block-diagonal
Wblk = diag(W, W) scaled by 1/sqrt(D).

q_r chunks are transposed two at a time ([128,128] PE transposes giving
chunk 2p at partitions 0..63 and 2p+1 at 64..127), and matmul B is 16
pair matmuls lhsT = qt_pair, rhs = Wblk -> out pair [128, (dv|dv)],
which is exactly the contiguous output layout.
"""

import math
import numpy as np

import concourse.bass as bass
import concourse.bacc as bacc
import concourse.mybir as mybir
import concourse.tile as tile
from concourse import bass_utils

B, S, D = 8, 4096, 64
P = 128            # SBUF partitions
R = S // P         # 32 tokens per partition
F = R * D          # 2048 floats per partition
H = F // 2         # rope group size (free elems)
N_CORES = 8
SCALE = 1.0 / math.sqrt(D)

F32 = mybir.dt.float32


def _rope_ops(eng, dst, scr, src, cs, sn, sl):
    """dst = src*cs + sw(src)*sn on free slice sl."""
    eng.tensor_mul(dst[:, sl], src[:, sl], cs[:, sl])
    sw = src[:, sl].rearrange("p (a l) -> p a l", l=2)[:, :, ::-1]
    snv = sn[:, sl].rearrange("p (a l) -> p a l", l=2)
    t2v = scr[:, sl].rearrange("p (a l) -> p a l", l=2)
    eng.tensor_mul(t2v, sw, snv)
    eng.tensor_add(dst[:, sl], dst[:, sl], scr[:, sl])


def _build(debug=False, n_tables=1):
    nc = bacc.Bacc(
        "TRN2", target_bir_lowering=False, debug=debug, num_devices=N_CORES
    )
    q = nc.dram_tensor("q", [P, F], F32, kind="ExternalInput")
    k = nc.dram_tensor("k", [P, F], F32, kind="ExternalInput")
    v = nc.dram_tensor("v", [P, F], F32, kind="ExternalInput")
    ident_d = nc.dram_tensor("ident", [P, P], F32, kind="ExternalInput")
    tabs = []
    for i in range(n_tables):
        sfx = "" if n_tables == 1 else f"_{i}"
        cs_d = nc.dram_tensor(f"cs{sfx}", [P, F], F32, kind="ExternalInput")
        sn_d = nc.dram_tensor(f"sn{sfx}", [P, F], F32, kind="ExternalInput")
        tabs.append((cs_d, sn_d))
    out_d = nc.dram_tensor("out", [P, F], F32, kind="ExternalOutput")

    with tile.TileContext(nc) as tc:
        with (
            tc.tile_pool(name="const", bufs=1) as const,
            tc.tile_pool(name="data", bufs=1) as data,
            tc.tile_pool(name="scratch", bufs=2) as scratch,
            tc.tile_pool(name="psA", bufs=1, space="PSUM") as psA,
            tc.tile_pool(name="psT", bufs=2, space="PSUM") as psT,
            tc.tile_pool(name="psO", bufs=2, space="PSUM") as psO,
        ):
            # ---- DMAs across both HWDGE rings (SP + ACT). The pieces
            # needed by the first rope op go first on the SP ring; ident
            # (only needed for transposes, late) goes last.
            Q1 = 512
            tabs_sb = []
            for i, (cs_d, sn_d) in enumerate(tabs):
                cs = const.tile([P, F], F32, tag=f"cs{i}")
                sn = const.tile([P, F], F32, tag=f"sn{i}")
                tabs_sb.append((cs, sn))
            k_sb = data.tile([P, F], F32, tag="k")
            v_sb = data.tile([P, F], F32, tag="v")
            q_sb = data.tile([P, F], F32, tag="q")
            ident = const.tile([P, P], F32, tag="ident")

            cs0, sn0 = tabs_sb[0]
            a, b = slice(0, Q1), slice(Q1, F)
            # Issue in exact consumption order, alternating rings so both
            # contribute bandwidth and no large transfer jumps the queue.
            ring = [nc.sync, nc.scalar]
            pieces = [
                (cs0, tabs[0][0], a), (sn0, tabs[0][1], a), (k_sb, k, a),
                (cs0, tabs[0][0], b), (sn0, tabs[0][1], b), (k_sb, k, b),
                (v_sb, v, slice(0, H)), (v_sb, v, slice(H, F)),
                (q_sb, q, slice(0, H)),
                (q_sb, q, slice(H, H + Q1)), (q_sb, q, slice(H + Q1, F)),
            ]
            for i, (sb, dr, sl) in enumerate(pieces):
                ring[i % 2].dma_start(out=sb[:, sl], in_=dr.ap()[:, sl])
            for i, (cs_d, sn_d) in enumerate(tabs[1:], 1):
                cs, sn = tabs_sb[i]
                nc.scalar.dma_start(out=cs[:], in_=cs_d.ap())
                nc.scalar.dma_start(out=sn[:], in_=sn_d.ap())
            nc.scalar.dma_start(out=ident[:], in_=ident_d.ap())

            k_r = data.tile([P, F], F32, tag="k_r")
            v_r = data.tile([P, F], F32, tag="v_r")
            q_r = data.tile([P, F], F32, tag="q_r")
            cs_q, sn_q = tabs_sb[0]
            cs_k, sn_k = tabs_sb[1 % n_tables]
            cs_v, sn_v = tabs_sb[2 % n_tables]
            scr = scratch.tile([P, F], F32, tag="t2")

            wbig = psA.tile([P, P], F32)

            def mmA(pr):  # one pair-packed accumulation step
                sl = slice(pr * P, (pr + 1) * P)
                nc.tensor.matmul(
                    wbig[:],
                    lhsT=k_r[:, sl],
                    rhs=v_r[:, sl],
                    start=(pr == 0),
                    stop=(pr == R // 2 - 1),
                )

            # ---- rope k/v group-interleaved with matmul A ----
            _rope_ops(nc.vector, k_r, scr, k_sb, cs_k, sn_k, slice(0, Q1))
            _rope_ops(nc.vector, k_r, scr, k_sb, cs_k, sn_k, slice(Q1, F))
            _rope_ops(nc.vector, v_r, scr, v_sb, cs_v, sn_v, slice(0, H))
            for pr in range(0, 8):
                mmA(pr)
            _rope_ops(nc.vector, v_r, scr, v_sb, cs_v, sn_v, slice(H, F))
            for pr in range(8, 16):
                mmA(pr)

            # ---- rope q group 0 (covers matmul-A latency on PE) ----
            qt_sb = data.tile([P, (R // 2) * P], F32, tag="qt")
            out_sb = data.tile([P, F], F32, tag="out")
            _rope_ops(nc.vector, q_r, scr, q_sb, cs_q, sn_q, slice(0, H))

            # ---- W = diag quadrants summed; Wblk = diag(W,W) * scale ----
            # (only one DVE input may be PSUM: stage quadrant 1 via ACT)
            w_b = data.tile([D, D], F32, tag="w_b")
            nc.scalar.mul(w_b[:], wbig[D:P, D:P], SCALE)
            wblk = data.tile([P, P], F32, tag="wblk")
            nc.scalar.memzero(wblk[:])
            nc.vector.scalar_tensor_tensor(
                wblk[0:D, 0:D], wbig[0:D, 0:D], SCALE, w_b[:],
                op0=mybir.AluOpType.mult, op1=mybir.AluOpType.add,
            )
            nc.scalar.copy(wblk[D:P, D:P], wblk[0:D, 0:D])

            def tr_block(t4):  # 4 transpose pairs into one PSUM bank
                t_ps = psT.tile([P, 4 * P], F32, tag="t_ps")
                for j in range(4):
                    pr = 4 * t4 + j
                    nc.tensor.transpose(
                        t_ps[:, j * P:(j + 1) * P],
                        q_r[:, pr * P:(pr + 1) * P],
                        ident[:],
                    )
                nc.scalar.copy(qt_sb[:, t4 * 4 * P:(t4 + 1) * 4 * P], t_ps[:])

            def mmB_block(t4):  # 4 pair matmuls into one PSUM bank
                o_ps = psO.tile([P, 4 * P], F32, tag="o_ps")
                for j in range(4):
                    pr = 4 * t4 + j
                    nc.tensor.matmul(
                        o_ps[:, j * P:(j + 1) * P],
                        lhsT=qt_sb[:, pr * P:(pr + 1) * P],
                        rhs=wblk[:],
                        start=(j == 0),
                        stop=(j == 3),
                    )
                nc.scalar.copy(out_sb[:, t4 * 4 * P:(t4 + 1) * 4 * P], o_ps[:])

            for t4 in (0, 1):
                tr_block(t4)
                mmB_block(t4)
            nc.sync.dma_start(out=out_d.ap()[:, 0:H], in_=out_sb[:, 0:H])
            # q rope tail in two quarter groups to shorten the PE tail
            for g3, t4 in ((slice(H, H + Q1), 2), (slice(H + Q1, F), 3)):
                _rope_ops(nc.vector, q_r, scr, q_sb, cs_q, sn_q, g3)
                tr_block(t4)
                mmB_block(t4)
                nc.sync.dma_start(out=out_d.ap()[:, g3], in_=out_sb[:, g3])

    nc.compile()
    return nc


_CACHE = {}


def _get_nc(debug=False, n_tables=1):
    key = (debug, n_tables)
    if key not in _CACHE:
        _CACHE[key] = _build(debug=debug, n_tables=n_tables)
    return _CACHE[key]


def _tables(freq):
    """freq [S, D//2, 2] (cos, sin) -> expanded (C, S') in div-layout."""
    f = np.asarray(freq, np.float32)
    cos, sin = f[..., 0], f[..., 1]          # [S, D//2]
    cs = np.repeat(cos, 2, axis=1)           # [S, D]
    sn = np.empty((S, D), np.float32)
    sn[:, 0::2] = -sin
    sn[:, 1::2] = sin
    return np.ascontiguousarray(cs.reshape(P, F)), sn.reshape(P, F)


def kernel(q, k, v, freq_q, freq_k, freq_v):
    q = np.ascontiguousarray(np.asarray(q, np.float32))
    k = np.ascontiguousarray(np.asarray(k, np.float32))
    v = np.ascontiguousarray(np.asarray(v, np.float32))
    fq = np.asarray(freq_q, np.float32)
    fk = np.asarray(freq_k, np.float32)
    fv = np.asarray(freq_v, np.float32)
    shared = np.array_equal(fq, fk) and np.array_equal(fq, fv)
    ident = np.eye(P, dtype=np.float32)

    if shared:
        nc = _get_nc(n_tables=1)
        cs, sn = _tables(fq)
        base = {"ident": ident, "cs": cs, "sn": sn}
    else:  # not produced by setup_inputs; correctness fallback
        nc = _get_nc(n_tables=3)
        base = {"ident": ident}
        for i, f in enumerate((fq, fk, fv)):
            cs, sn = _tables(f)
            base[f"cs_{i}"] = cs
            base[f"sn_{i}"] = sn

    in_maps = []
    for b in range(B):
        m = dict(base)
        m["q"] = q[b].reshape(P, F)
        m["k"] = k[b].reshape(P, F)
        m["v"] = v[b].reshape(P, F)
        in_maps.append(m)

    res = bass_utils.run_bass_kernel_spmd(nc, in_maps, core_ids=list(range(N_CORES)))
    out = np.stack([res.results[b]["out"].reshape(S, D) for b in range(B)])
    return out


# revision 13
# speedup vs baseline: 1.0039x; 1.0039x over previous
"""MultiHeadRoPE kernel for Trainium2 (8 NeuronCores, batch-sharded).

Computes, per batch b:
    q_r = rope(q[b]); k_r = rope(k[b]); v_r = rope(v[b])
    out[b] = (q_r @ k_r.T) * (1/sqrt(D)) @ v_r
reassociated (no softmax present) as
    out[b] = q_r @ (scale * (k_r.T @ v_r))
turning an O(S^2 D) problem with a 64MB intermediate into an O(S D^2)
memory-bound one.

Device layout ("div-layout"): a [S, D] = [4096, 64] tensor is stored in
SBUF/DRAM as [128, 2048] where partition p holds tokens 32p..32p+31
contiguously -> every DMA moves 8KB contiguous per partition.

RoPE: out = x*C + sw(x)*S' (sw = pair swap; S' sign-folded). All rope on
DVE only: GPSIMD shares DVE's second SBUF port, so concurrent gpsimd
elementwise halves both engines. Work is split into 2 groups per tensor
and DMAs are split to match, so PE matmuls pipeline behind rope groups.

Matmul A (W = k_r.T @ v_r) runs as 16 pair-packed [M=128,K=128,N=128]
fp32 matmuls: lhsT = [k_r(2p) | k_r(2p+1)], rhs = [v_r(2p) | v_r(2p+1)]
accumulating into one [128,128] PSUM tile whose diagonal 64x64 quadrants
hold the two true partial products (off-diagonal quadrants are unused
cross terms). A DVE add folds the quadrants; ACT builds a block-diagonal
Wblk = diag(W, W) scaled by 1/sqrt(D).

q_r chunks are transposed two at a time ([128,128] PE transposes giving
chunk 2p at partitions 0..63 and 2p+1 at 64..127), and matmul B is 16
pair matmuls lhsT = qt_pair, rhs = Wblk -> out pair [128, (dv|dv)],
which is exactly the contiguous output layout.
"""

import math
import numpy as np

import concourse.bass as bass
import concourse.bacc as bacc
import concourse.mybir as mybir
import concourse.tile as tile
from concourse import bass_utils

B, S, D = 8, 4096, 64
P = 128            # SBUF partitions
R = S // P         # 32 tokens per partition
F = R * D          # 2048 floats per partition
H = F // 2         # rope group size (free elems)
N_CORES = 8
SCALE = 1.0 / math.sqrt(D)

F32 = mybir.dt.float32


def _rope_ops(eng, dst, scr, src, cs, sn, sl):
    """dst = src*cs + sw(src)*sn on free slice sl."""
    eng.tensor_mul(dst[:, sl], src[:, sl], cs[:, sl])
    sw = src[:, sl].rearrange("p (a l) -> p a l", l=2)[:, :, ::-1]
    snv = sn[:, sl].rearrange("p (a l) -> p a l", l=2)
    t2v = scr[:, sl].rearrange("p (a l) -> p a l", l=2)
    eng.tensor_mul(t2v, sw, snv)
    eng.tensor_add(dst[:, sl], dst[:, sl], scr[:, sl])


def _build(debug=False, n_tables=1):
    nc = bacc.Bacc(
        "TRN2", target_bir_lowering=False, debug=debug, num_devices=N_CORES
    )
    q = nc.dram_tensor("q", [P, F], F32, kind="ExternalInput")
    k = nc.dram_tensor("k", [P, F], F32, kind="ExternalInput")
    v = nc.dram_tensor("v", [P, F], F32, kind="ExternalInput")
    ident_d = nc.dram_tensor("ident", [P, P], F32, kind="ExternalInput")
    tabs = []
    for i in range(n_tables):
        sfx = "" if n_tables == 1 else f"_{i}"
        cs_d = nc.dram_tensor(f"cs{sfx}", [P, F], F32, kind="ExternalInput")
        sn_d = nc.dram_tensor(f"sn{sfx}", [P, F], F32, kind="ExternalInput")
        tabs.append((cs_d, sn_d))
    out_d = nc.dram_tensor("out", [P, F], F32, kind="ExternalOutput")

    with tile.TileContext(nc) as tc:
        with (
            tc.tile_pool(name="const", bufs=1) as const,
            tc.tile_pool(name="data", bufs=1) as data,
            tc.tile_pool(name="scratch", bufs=2) as scratch,
            tc.tile_pool(name="psA", bufs=1, space="PSUM") as psA,
            tc.tile_pool(name="psT", bufs=3, space="PSUM") as psT,
            tc.tile_pool(name="psO", bufs=3, space="PSUM") as psO,
        ):
            # ---- DMAs across both HWDGE rings (SP + ACT). The pieces
            # needed by the first rope op go first on the SP ring; ident
            # (only needed for transposes, late) goes last.
            Q1 = 512
            tabs_sb = []
            for i, (cs_d, sn_d) in enumerate(tabs):
                cs = const.tile([P, F], F32, tag=f"cs{i}")
                sn = const.tile([P, F], F32, tag=f"sn{i}")
                tabs_sb.append((cs, sn))
            k_sb = data.tile([P, F], F32, tag="k")
            v_sb = data.tile([P, F], F32, tag="v")
            q_sb = data.tile([P, F], F32, tag="q")
            ident = const.tile([P, P], F32, tag="ident")

            cs0, sn0 = tabs_sb[0]
            a, b = slice(0, Q1), slice(Q1, F)
            # Single ring (SP) in exact consumption order: per-ring FIFO is
            # the only way to give early pieces true priority (all queued
            # DMAs otherwise share SDMA bandwidth round-robin). The ACT
            # ring carries only ident / extra tables / output stores.
            pieces = [
                (cs0, tabs[0][0], a), (sn0, tabs[0][1], a), (k_sb, k, a),
                (cs0, tabs[0][0], b), (sn0, tabs[0][1], b), (k_sb, k, b),
                (v_sb, v, slice(0, H)), (v_sb, v, slice(H, F)),
                (q_sb, q, slice(0, H)),
                (q_sb, q, slice(H, H + Q1)), (q_sb, q, slice(H + Q1, F)),
            ]
            for sb, dr, sl in pieces:
                nc.sync.dma_start(out=sb[:, sl], in_=dr.ap()[:, sl])
            for i, (cs_d, sn_d) in enumerate(tabs[1:], 1):
                cs, sn = tabs_sb[i]
                nc.scalar.dma_start(out=cs[:], in_=cs_d.ap())
                nc.scalar.dma_start(out=sn[:], in_=sn_d.ap())
            nc.scalar.dma_start(out=ident[:], in_=ident_d.ap())

            k_r = data.tile([P, F], F32, tag="k_r")
            v_r = data.tile([P, F], F32, tag="v_r")
            q_r = data.tile([P, F], F32, tag="q_r")
            cs_q, sn_q = tabs_sb[0]
            cs_k, sn_k = tabs_sb[1 % n_tables]
            cs_v, sn_v = tabs_sb[2 % n_tables]
            scr = scratch.tile([P, F], F32, tag="t2")

            wbig = psA.tile([P, P], F32)

            def mmA(pr):  # one pair-packed accumulation step
                sl = slice(pr * P, (pr + 1) * P)
                nc.tensor.matmul(
                    wbig[:],
                    lhsT=k_r[:, sl],
                    rhs=v_r[:, sl],
                    start=(pr == 0),
                    stop=(pr == R // 2 - 1),
                )

            # ---- rope k/v group-interleaved with matmul A ----
            _rope_ops(nc.vector, k_r, scr, k_sb, cs_k, sn_k, slice(0, Q1))
            _rope_ops(nc.vector, k_r, scr, k_sb, cs_k, sn_k, slice(Q1, F))
            _rope_ops(nc.vector, v_r, scr, v_sb, cs_v, sn_v, slice(0, H))
            for pr in range(0, 8):
                mmA(pr)
            _rope_ops(nc.vector, v_r, scr, v_sb, cs_v, sn_v, slice(H, F))
            for pr in range(8, 16):
                mmA(pr)

            # ---- rope q group 0 (covers matmul-A latency on PE) ----
            qt_sb = data.tile([P, (R // 2) * P], F32, tag="qt")
            out_sb = data.tile([P, F], F32, tag="out")
            _rope_ops(nc.vector, q_r, scr, q_sb, cs_q, sn_q, slice(0, H))

            # ---- W = diag quadrants summed; Wblk = diag(W,W) * scale ----
            # (only one DVE input may be PSUM: stage quadrant 1 via ACT)
            w_b = data.tile([D, D], F32, tag="w_b")
            nc.scalar.mul(w_b[:], wbig[D:P, D:P], SCALE)
            wblk = data.tile([P, P], F32, tag="wblk")
            nc.scalar.memzero(wblk[:])
            nc.vector.scalar_tensor_tensor(
                wblk[0:D, 0:D], wbig[0:D, 0:D], SCALE, w_b[:],
                op0=mybir.AluOpType.mult, op1=mybir.AluOpType.add,
            )
            nc.scalar.copy(wblk[D:P, D:P], wblk[0:D, 0:D])

            def tr_block(p0, n):  # n transpose pairs into one PSUM bank
                t_ps = psT.tile([P, 4 * P], F32, tag="t_ps")
                for j in range(n):
                    pr = p0 + j
                    nc.tensor.transpose(
                        t_ps[:, j * P:(j + 1) * P],
                        q_r[:, pr * P:(pr + 1) * P],
                        ident[:],
                    )
                nc.scalar.copy(
                    qt_sb[:, p0 * P:(p0 + n) * P], t_ps[:, 0:n * P]
                )

            def mmB_block(p0, n):  # n pair matmuls into one PSUM bank
                o_ps = psO.tile([P, 4 * P], F32, tag="o_ps")
                for j in range(n):
                    pr = p0 + j
                    nc.tensor.matmul(
                        o_ps[:, j * P:(j + 1) * P],
                        lhsT=qt_sb[:, pr * P:(pr + 1) * P],
                        rhs=wblk[:],
                        start=(j == 0),
                        stop=(j == n - 1),
                    )
                nc.scalar.copy(
                    out_sb[:, p0 * P:(p0 + n) * P], o_ps[:, 0:n * P]
                )

            for p0 in (0, 4):
                tr_block(p0, 4)
                mmB_block(p0, 4)
            nc.scalar.dma_start(out=out_d.ap()[:, 0:H], in_=out_sb[:, 0:H])
            # q rope tail in shrinking groups to shorten the PE tail
            tail = [
                (slice(H, H + Q1), [(8, 4)]),
                (slice(H + Q1, F - 256), [(12, 2)]),
                (slice(F - 256, F), [(14, 2)]),
            ]
            for g3, blocks in tail:
                _rope_ops(nc.vector, q_r, scr, q_sb, cs_q, sn_q, g3)
                for p0, n in blocks:
                    tr_block(p0, n)
                    mmB_block(p0, n)
                nc.scalar.dma_start(out=out_d.ap()[:, g3], in_=out_sb[:, g3])

    nc.compile()
    return nc


_CACHE = {}


def _get_nc(debug=False, n_tables=1):
    key = (debug, n_tables)
    if key not in _CACHE:
        _CACHE[key] = _build(debug=debug, n_tables=n_tables)
    return _CACHE[key]


def _tables(freq):
    """freq [S, D//2, 2] (cos, sin) -> expanded (C, S') in div-layout."""
    f = np.asarray(freq, np.float32)
    cos, sin = f[..., 0], f[..., 1]          # [S, D//2]
    cs = np.repeat(cos, 2, axis=1)           # [S, D]
    sn = np.empty((S, D), np.float32)
    sn[:, 0::2] = -sin
    sn[:, 1::2] = sin
    return np.ascontiguousarray(cs.reshape(P, F)), sn.reshape(P, F)


def kernel(q, k, v, freq_q, freq_k, freq_v):
    q = np.ascontiguousarray(np.asarray(q, np.float32))
    k = np.ascontiguousarray(np.asarray(k, np.float32))
    v = np.ascontiguousarray(np.asarray(v, np.float32))
    fq = np.asarray(freq_q, np.float32)
    fk = np.asarray(freq_k, np.float32)
    fv = np.asarray(freq_v, np.float32)
    shared = np.array_equal(fq, fk) and np.array_equal(fq, fv)
    ident = np.eye(P, dtype=np.float32)

    if shared:
        nc = _get_nc(n_tables=1)
        cs, sn = _tables(fq)
        base = {"ident": ident, "cs": cs, "sn": sn}
    else:  # not produced by setup_inputs; correctness fallback
        nc = _get_nc(n_tables=3)
        base = {"ident": ident}
        for i, f in enumerate((fq, fk, fv)):
            cs, sn = _tables(f)
            base[f"cs_{i}"] = cs
            base[f"sn_{i}"] = sn

    in_maps = []
    for b in range(B):
        m = dict(base)
        m["q"] = q[b].reshape(P, F)
        m["k"] = k[b].reshape(P, F)
        m["v"] = v[b].reshape(P, F)
        in_maps.append(m)

    res = bass_utils.run_bass_kernel_spmd(nc, in_maps, core_ids=list(range(N_CORES)))
    out = np.stack([res.results[b]["out"].reshape(S, D) for b in range(B)])
    return out
